# revision 15
# baseline (speedup 1.0000x reference)
"""DigitCaps routing kernel v2 for 8 Trainium2 NeuronCores.

Math (b-logits start at 0; O_t = sum of squash outputs so far):
  u[b,i,c,o] = sum_v W[i,c,o,v] x[b,i,v]
  iter t>0:  l = u*O_{t-1}; e = exp(l-5); den = sum_c e; s = sum_i (e/den)*u
  iter 0:    s = sum_i u / 64
  out_t = squash(s); final answer = out_2.

v2 design vs v1 baseline:
  - moving-operand columns per matmul are (b8, il16) [b-outer] so per-b
    slices of psum/u_t are i-contiguous -> tensor_scalar 4x mode for l=u*O
    with a per-partition O column (O is per-partition in the (c_low,o)
    layout once b and ch are fixed).
  - u_t (fp16 [128, (b8,i256)] per (G,ch)) is materialized once (iter 1),
    round-tripped through DRAM, and re-streamed for iter 2: iter 2 has no
    matmuls and no PSUM evacuation.
  - s-reduce: stt(eh, 1.0, u_t) with accum_out (no invO identity, no
    O-clamping, no SCALE) -> simpler squash, better precision.
  - eh = e*inv on DVE (fp16 2x) with optional Pool offload.
  - PSUM evacuation split between ACT (activation Copy) and DVE.
  - activation tables pinned to the natural_log_exp_and_others set so the
    whole kernel does exactly one ACT table load.
"""
import os
import sys

sys.path.insert(0, "/opt/trn_rl_repo")

import numpy as np

_CACHE = {}

BS, NI, NC_, OL, NV = 64, 2048, 64, 16, 8
CO = NC_ * OL          # 1024
NCORES = 8
ILOC = NI // NCORES    # 256
NG = 16                # i-groups per core (16 i each)
EPS = 1e-9
EB = 5.0               # exp bias

# knobs
ACT_EVAC = int(os.environ.get("KACT_EVAC", "16"))    # of 16 (ch,H): evac on ACT
POOL_EH = int(os.environ.get("KPOOL_EH", "0"))      # of 16 (ch,Q): eh on Pool
AR16 = int(os.environ.get("KAR16", "1"))            # fp16 allreduce staging
PIN_ACT = int(os.environ.get("KPIN_ACT", "1"))      # pin one act table set


def _pin_act_tables():
    """Restrict the activation-table sets so every func this kernel uses
    resolves to natural_log_exp_and_others -> exactly one table load."""
    from concourse import hw_specs
    import concourse.mybir as mybir
    tabs = hw_specs.get_activation_tables("gen3")
    AF = mybir.ActivationFunctionType
    used = {AF.Exp, AF.Ln, AF.Copy, AF.Identity, AF.Abs, AF.Sign, AF.Square}
    for name, s in tabs.items():
        if name != "natural_log_exp_and_others":
            s -= used
    return tabs


def _build_program(reps=1, ncores=NCORES, use_collective=True):
    import concourse.bass as bass
    import concourse.bacc as bacc
    import concourse.mybir as mybir
    import concourse.tile as tile

    F32 = mybir.dt.float32
    FP16 = mybir.dt.float16
    AO = mybir.AluOpType
    AF = mybir.ActivationFunctionType

    if PIN_ACT:
        _pin_act_tables()

    nc = bacc.Bacc("TRN2", target_bir_lowering=False, debug=False,
                   num_devices=ncores)

    wb_in = nc.dram_tensor("wb", [128, NG * CO], FP16, kind="ExternalInput")
    xz_in = nc.dram_tensor("xz", [128, NG * 8 * 8 * 16], FP16,
                           kind="ExternalInput")
    xd_in = nc.dram_tensor("xd", [128, NG * BS], FP16, kind="ExternalInput")
    ones_in = nc.dram_tensor("ones", [128, 128], FP16, kind="ExternalInput")
    # out layout [p=(c_low8, o16), (ch8, b64)]; host transposes to [b, c, o]
    out_d = nc.dram_tensor("out", [128, 8 * BS], F32, kind="ExternalOutput")

    with tile.TileContext(nc) as tc:
        with (
            tc.tile_pool(name="big", bufs=1) as big,
            tc.tile_pool(name="upool", bufs=10) as upool,
            tc.tile_pool(name="lpool", bufs=3) as lpool,
            tc.tile_pool(name="epool", bufs=10) as epool,
            tc.tile_pool(name="hpool", bufs=2) as hpool,
            tc.tile_pool(name="ipool", bufs=4) as ipool,
            tc.tile_pool(name="small", bufs=1) as small,
            tc.tile_pool(name="lnp", bufs=2) as lnp,
            tc.tile_pool(name="scr", bufs=4) as scr,
            tc.tile_pool(name="psu_p", bufs=2, space="PSUM") as psu_p,
            tc.tile_pool(name="psd_p", bufs=2, space="PSUM") as psd_p,
            tc.tile_pool(name="dram", bufs=1, space="DRAM") as dram,
        ):
          u_d = dram.tile([128, 64 * 2048], FP16, tag="u_d")
          cc_dt = FP16 if AR16 else F32
          cc_in0 = dram.tile([CO, BS // 2], cc_dt, tag="cc_in0")
          cc_in1 = dram.tile([CO, BS // 2], cc_dt, tag="cc_in1")
          cc_out0 = dram.tile([CO, BS // 2], cc_dt, tag="cc_out0")
          cc_out1 = dram.tile([CO, BS // 2], cc_dt, tag="cc_out1")
          cc_in = [cc_in0, cc_in1]
          cc_out = [cc_out0, cc_out1]
          for rep in range(reps):
            wb = big.tile([128, NG * CO], FP16, tag="wb")
            xz = big.tile([128, NG * 8 * 8 * 16], FP16, tag="xz")
            xd = big.tile([128, NG * BS], FP16, tag="xd")
            ones_oc = small.tile([128, 128], FP16, tag="ones")
            nc.sync.dma_start(xd[:], xd_in[:])
            nc.sync.dma_start(ones_oc[:], ones_in[:])
            # chunked so the first matmuls (low g) start early
            Q4 = NG * CO // 4
            for c4 in range(4):
                eng = (nc.sync, nc.scalar)[c4 % 2]
                eng.dma_start(wb[:, c4 * Q4:(c4 + 1) * Q4],
                              wb_in[:, c4 * Q4:(c4 + 1) * Q4])
            X4 = NG * 8 * 8 * 16 // 4
            for c4 in range(4):
                eng = (nc.scalar, nc.gpsimd)[c4 % 2]
                eng.dma_start(xz[:, c4 * X4:(c4 + 1) * X4],
                              xz_in[:, c4 * X4:(c4 + 1) * X4])

            # wb view: [128, g, co]; xz view: [128, g, G, (b il)]
            wb_v = wb[:].rearrange("p (g co) -> p g co", g=NG)
            xz_v = xz[:].rearrange("p (g G n) -> p g G n", g=NG, G=8)
            xd_v = xd[:].rearrange("p (g b) -> p g b", g=NG)

            biasb = small.tile([128, 1], F32, tag="biasb")
            nc.vector.memset(biasb[:], -EB)
            # Boundary state split into independent b-halves (b-octets 0-3 /
            # 4-7) so the half-AllReduce + half-squash chains pipeline: the
            # first half's AR fires mid-iteration (after G=3's s-reduce) and
            # the next iteration's first G-groups only depend on half 0.
            HB = BS // 2  # 32 b per half
            s_loc, sr_sb, O_sb, t1, t2, t3 = [], [], [], [], [], []
            for h in range(2):
                for lst, nm in ((s_loc, "sloc"), (sr_sb, "sr"), (O_sb, "O"),
                                (t1, "t1"), (t2, "t2"), (t3, "t3")):
                    tl = small.tile([128, 8 * HB], F32, tag=f"{nm}{h}")
                    lst.append(tl)

            def s_col(G, ch, b):
                """accum_out slot for (G, ch, b)."""
                h = G // 4
                return (s_loc[h][:].rearrange("p (ch b) -> p ch b", ch=8)
                        [:, ch, (G % 4) * 8 + b:(G % 4) * 8 + b + 1])

            def O_col(G, ch, b):
                h = G // 4
                return (O_sb[h][:].rearrange("p (ch b) -> p ch b", ch=8)
                        [:, ch, (G % 4) * 8 + b:(G % 4) * 8 + b + 1])

            def allreduce_s(h):
                cin = cc_in[h][:].rearrange("(ch p) b -> p ch b", p=128)
                sv = s_loc[h][:].rearrange("p (ch b) -> p ch b", ch=8)
                # SWDGE path casts f32 -> fp16 during staging when AR16
                eng = nc.gpsimd if AR16 else nc.sync
                eng.dma_start(cin[:], sv[:])
                if use_collective:
                    nc.gpsimd.collective_compute(
                        "AllReduce",
                        AO.add,
                        replica_groups=[list(range(ncores))],
                        ins=[cc_in[h].opt()],
                        outs=[cc_out[h].opt()],
                    )
                    src = cc_out[h]
                else:
                    src = cc_in[h]
                srv = sr_sb[h][:].rearrange("p (ch b) -> p ch b", ch=8)
                cov = src[:].rearrange("(ch p) b -> p ch b", p=128)
                eng.dma_start(srv[:], cov[:])

            def squash_and_accum(h, first: bool, last: bool):
                """out_t = squash(sr[h]); O[h] += out_t. Last: DMA out."""
                V = sr_sb[h][:]
                a1, a2, a3 = t1[h][:], t2[h][:], t3[h][:]
                nc.vector.tensor_tensor(a1, V, V, op=AO.mult)   # s^2
                nc.scalar.activation(a2, V, AF.Abs)             # |s|
                # a3 = (|s|+eps)*(1+s^2)
                nc.vector.tensor_scalar(out=a1, in0=a1, scalar1=1.0,
                                        scalar2=None, op0=AO.add)
                nc.vector.scalar_tensor_tensor(
                    a3, a2, EPS, a1, op0=AO.add, op1=AO.mult)
                # a3 = 1/a3 (exp(-ln))
                nc.scalar.activation(a2, a3, AF.Ln)
                nc.scalar.activation(a3, a2, AF.Exp, scale=-1.0)
                # a1 = 1+s^2 -> s^2 = a1-1
                nc.vector.tensor_scalar(out=a1, in0=a1, scalar1=-1.0,
                                        scalar2=None, op0=AO.add)
                nc.vector.tensor_tensor(a2, V, a1, op=AO.mult)  # s^3
                nc.vector.tensor_tensor(a2, a2, a3, op=AO.mult)
                if last:
                    # out cols (ch, b): half h covers b-octets 4h..4h+3,
                    # i.e. cols (ch, 32h..32h+32)
                    ov = out_d[:].rearrange("p (ch b) -> p ch b", ch=8)
                    a2v = t2[h][:].rearrange("p (ch b) -> p ch b", ch=8)
                    eng = nc.sync if h == 0 else nc.scalar
                    eng.dma_start(ov[:, :, 32 * h:32 * h + 32], a2v[:])
                    return
                if first:
                    nc.vector.tensor_copy(O_sb[h][:], a2)
                else:
                    nc.vector.tensor_tensor(O_sb[h][:], O_sb[h][:], a2,
                                            op=AO.add)

            # ---------------- iter 0: s0 = (1/64) * sum_i u ----------------
            for ch in range(8):
                ps0 = psu_p.tile([128, 1024], F32, tag="psu")
                for g in range(NG):
                    nc.tensor.matmul(
                        ps0[:, :BS],
                        wb_v[:, g, ch * 128:(ch + 1) * 128],
                        xd_v[:, g, :],
                        start=(g == 0),
                        stop=(g == NG - 1),
                    )
                for h in range(2):
                    nc.vector.tensor_scalar(
                        out=s_loc[h][:].rearrange("p (ch b) -> p ch b",
                                                  ch=8)[:, ch, :],
                        in0=ps0[:, 32 * h:32 * h + 32], scalar1=1.0 / NC_,
                        scalar2=None, op0=AO.mult)
            for h in range(2):
                allreduce_s(h)
            for h in range(2):
                squash_and_accum(h, first=True, last=False)

            # ---------------- iters 1, 2 ----------------
            for it in (1, 2):
                state = {}

                def front(G, ch):
                    st = state.setdefault(G, {"u": {}, "e": {}, "psd": []})
                    u_t = upool.tile([128, 2048], FP16, tag="u")
                    st["u"][ch] = u_t
                    uv = u_t[:].rearrange(
                        "p (b H gg il) -> p b H gg il", b=8, H=2, gg=8)
                    if it == 1:
                        for H in range(2):
                            psu = psu_p.tile([128, 1024], F32, tag="psu")
                            for gg in range(8):
                                g = 8 * H + gg
                                nc.tensor.matmul(
                                    psu[:, gg * 128:(gg + 1) * 128],
                                    wb_v[:, g, ch * 128:(ch + 1) * 128],
                                    xz_v[:, g, G, :],
                                    start=True, stop=True,
                                )
                            # evacuate psum (cols (gg, b, il)) into u_t
                            # (cols (b, H, gg, il)), fp16. ch0 (the G+1
                            # lookahead unit) evacuates on DVE so it can
                            # fill DVE's inv-wait bubble without queueing
                            # behind ACT's exp/inv chain.
                            psu_v = psu[:].rearrange(
                                "p (gg b il) -> p b gg il", gg=8, b=8)
                            on_act = (ch != 0
                                      and (2 * ch + H) % 16 < ACT_EVAC)
                            if on_act:
                                nc.scalar.activation(
                                    uv[:, :, H], psu_v, AF.Copy)
                            else:
                                nc.vector.tensor_copy(uv[:, :, H], psu_v)
                        # persist u_t for iter 2
                        nc.sync.dma_start(
                            u_d[:, (G * 8 + ch) * 2048:
                                (G * 8 + ch + 1) * 2048],
                            u_t[:])
                    else:
                        nc.scalar.dma_start(
                            u_t[:],
                            u_d[:, (G * 8 + ch) * 2048:
                                (G * 8 + ch + 1) * 2048])
                    # l = u * O  (per-b tensor_scalar, 4x mode)
                    l_t = lpool.tile([128, 2048], FP16, tag="l")
                    for b in range(8):
                        nc.vector.tensor_scalar(
                            out=l_t[:, b * 256:(b + 1) * 256],
                            in0=u_t[:, b * 256:(b + 1) * 256],
                            scalar1=O_col(G, ch, b),
                            scalar2=None, op0=AO.mult)
                    # e = exp(l - EB)
                    e_t = epool.tile([128, 2048], FP16, tag="e")
                    st["e"][ch] = e_t
                    nc.scalar.activation(e_t[:], l_t[:], AF.Exp,
                                         bias=biasb[:])
                    # den: psd[p', col] += sum_p ones[p,p'] e[p,col]
                    for Q in range(2):
                        if ch == 0:
                            psd = psd_p.tile([128, 1024], F32, tag="psd")
                            st["psd"].append(psd)
                        psd = st["psd"][Q]
                        for q in range(2):
                            nc.tensor.matmul(
                                psd[:, q * 512:(q + 1) * 512],
                                ones_oc[:],
                                e_t[:, Q * 1024 + q * 512:
                                    Q * 1024 + (q + 1) * 512],
                                start=(ch == 0), stop=(ch == 7),
                            )

                def back(G):
                    st = state[G]
                    # inv = 1/den via exp(-ln(den)), c-replicated.
                    # Q-interleaved so eh(Q0) starts before Q1's chain.
                    inv_tiles = []
                    for Q in range(2):
                        ln_t = lnp.tile([128, 1024], F32, tag="lnden")
                        nc.scalar.activation(ln_t[:], st["psd"][Q][:], AF.Ln)
                        inv_t = ipool.tile([128, 1024], FP16, tag="inv")
                        nc.scalar.activation(inv_t[:], ln_t[:], AF.Exp,
                                             scale=-1.0)
                        inv_tiles.append(inv_t)
                    for ch in range(8):
                        u_t, e_t = st["u"][ch], st["e"][ch]
                        eh_t = hpool.tile([128, 2048], FP16, tag="eh")
                        for Q in range(2):
                            heng = (nc.gpsimd
                                    if (2 * ch + Q) % 16 < POOL_EH
                                    else nc.vector)
                            heng.tensor_tensor(
                                eh_t[:, Q * 1024:(Q + 1) * 1024],
                                e_t[:, Q * 1024:(Q + 1) * 1024],
                                inv_tiles[Q][:],
                                op=AO.mult)
                        # s[:, ch, G*8+b] = sum_i eh*u
                        for b in range(8):
                            sc = scr.tile([128, 256], FP16, tag="scr")
                            nc.vector.scalar_tensor_tensor(
                                sc[:],
                                eh_t[:, b * 256:(b + 1) * 256],
                                1.0,
                                u_t[:, b * 256:(b + 1) * 256],
                                op0=AO.mult, op1=AO.mult,
                                accum_out=s_col(G, ch, b),
                            )
                    del state[G]

                for G in range(8):
                    for ch in range(8):
                        if ch not in state.get(G, {}).get("u", {}):
                            front(G, ch)
                    # 1-unit lookahead: emit G+1's first front before G's
                    # back phase so DVE has independent work queued while
                    # it waits for inv(G).
                    if G < 7:
                        front(G + 1, 0)
                    back(G)
                    # half-0's s columns (b-octets 0-3) are final after G=3:
                    # fire its AllReduce + squash mid-iteration so iter t+1's
                    # first G-groups don't wait for half 1.
                    if G == 3:
                        allreduce_s(0)
                        squash_and_accum(0, first=False, last=(it == 2))
                allreduce_s(1)
                squash_and_accum(1, first=False, last=(it == 2))

    nc.compile()
    return nc


def _prep_inputs(inputs, W):
    """Slice + relayout per core (fp16). Returns in_maps list."""
    x = np.asarray(inputs, dtype=np.float32)
    W = np.asarray(W, dtype=np.float32)
    ones = np.zeros((128, 128), np.float16)
    ones[np.arange(128)[:, None] % 16 == np.arange(128)[None, :] % 16] = 1
    in_maps = []
    for k in range(NCORES):
        i0 = k * ILOC
        Wk = W[i0:i0 + ILOC]                      # [256, 64, 16, 8]
        Wr = Wk.reshape(NG, 16, NC_, OL, NV)
        wb = np.ascontiguousarray(
            Wr.transpose(1, 4, 0, 2, 3)).reshape(128, NG * CO).astype(np.float16)
        xk = x[:, i0:i0 + ILOC, :]                # [64, 256, 8]
        xr = xk.reshape(8, 8, NG, 16, NV)          # G b g il v
        # xd[il*8+v, (g, G*8+b)] = x[G*8+b, 16g+il, v]  (dense, iter0)
        xd = np.ascontiguousarray(
            xr.transpose(3, 4, 2, 0, 1)).reshape(128, NG * BS).astype(np.float16)
        # xz[il*8+v, (g, G, b, il')] = x[.] if il==il' else 0  (b-outer!)
        xt = xr.transpose(3, 4, 2, 0, 1)           # [il, v, g, G, b]
        xz = np.zeros((16, 8, NG, 8, 8, 16), np.float32)
        for il in range(16):
            xz[il, :, :, :, :, il] = xt[il]
        xz = np.ascontiguousarray(
            xz.reshape(128, NG * 8 * 8 * 16)).astype(np.float16)
        in_maps.append({"wb": wb, "xz": xz, "xd": xd, "ones": ones})
    return in_maps


def kernel(inputs, W):
    from concourse import bass_utils

    if "nc" not in _CACHE:
        _CACHE["nc"] = _build_program()
    nc = _CACHE["nc"]
    in_maps = _prep_inputs(inputs, W)
    res = bass_utils.run_bass_kernel_spmd(nc, in_maps, list(range(NCORES)))
    out = np.asarray(res.results[0]["out"], dtype=np.float32)
    # out[p=(c_low8, o16), (ch8, b64)] -> [b, c=ch*8+c_low, o]
    out = out.reshape(8, OL, 8, BS).transpose(3, 2, 0, 1)
    return np.ascontiguousarray(out).reshape(BS, NC_, OL)
